# revision 5
# baseline (speedup 1.0000x reference)
"""Trainium2 Bass kernel: batch row-sharded grouped GEMM (MoE routing).

Contract: kernel(x, weight, num_inputs_per_group) takes FULL inputs
  x (32768, 2048) f32, weight (16, 2048, 2048) f32, num_inputs_per_group (16,) i32
and returns the FULL output (32768, 2048) f32, where token row i is multiplied
by weight[seg[i]] with seg = repeat(arange(16), num, total_repeat_length=32768)
(contiguous groups).

Distribution strategy (no collectives needed): tokens are split into contiguous
128-row blocks; each of the 8 cores gets an equal number of blocks plus the
weights for the experts its blocks use (expert/token parallelism — sanctioned
by the sharding hint since E=16 >= 8). Each core computes a dense grouped GEMM
locally and the host concatenates the per-core outputs.

Device kernel: fp32r matmuls (TF32-like input rounding, fp32 accumulation,
full PE rate), weights cached in SBUF per expert run, x pre-transposed on the
host so every DMA is wide and contiguous.
"""

import sys

sys.path.insert(0, "/opt/trn_rl_repo")

import numpy as np

import concourse.bacc as bacc
import concourse.mybir as mybir
from concourse.bass_utils import run_bass_kernel_spmd
from concourse.tile import TileContext

N_TOK, D_IN, D_OUT, N_EXP = 32768, 2048, 2048, 16
NCORES = 8
PB = 128  # token block = PSUM partition count
NT = 512  # matmul moving free dim = one fp32 PSUM bank
KT = D_IN // PB  # 16 k-tiles
NTILES = D_OUT // NT  # 4 output column tiles
MG_BLOCKS = 2  # token blocks per xT group DMA (256 tokens -> 1KB DMA chunks)

# Introspection hooks for test.py (harness just calls kernel()).
TRACE = False
LAST_RESULTS = None


def _seg_from_groups(num):
    """Replicate jnp.repeat(arange(E), num, total_repeat_length=N) semantics."""
    num = np.asarray(num, dtype=np.int64)
    reps = np.repeat(np.arange(N_EXP, dtype=np.int32), np.maximum(num, 0))
    if len(reps) >= N_TOK:
        return reps[:N_TOK]
    pad = reps[-1] if len(reps) else np.int32(0)
    return np.concatenate([reps, np.full(N_TOK - len(reps), pad, np.int32)])


def _build_nc(n_blocks_core, runs, n_slots):
    """Build the per-core SPMD kernel.

    runs: list of (slot, n_blocks) with sum(n_blocks) == n_blocks_core.
    Every core runs this same program; per-core data (x slice, slot->expert
    weight choice) lives in the input maps.
    """
    T_core = n_blocks_core * PB
    f32 = mybir.dt.float32
    f32r = mybir.dt.float32r

    nc = bacc.Bacc("TRN2", target_bir_lowering=False, debug=False, num_devices=NCORES)
    xT = nc.dram_tensor("xT", [D_IN, T_core], f32r, kind="ExternalInput")
    w = nc.dram_tensor("w", [n_slots, D_IN, D_OUT], f32r, kind="ExternalInput")
    out = nc.dram_tensor("out", [T_core, D_OUT], f32, kind="ExternalOutput")

    with TileContext(nc) as tc:
        with (
            tc.tile_pool(name="wpool", bufs=68) as wpool,
            tc.tile_pool(name="xpool", bufs=2) as xpool,
            tc.tile_pool(name="opool", bufs=2) as opool,
            tc.tile_pool(name="pspool", bufs=8, space="PSUM") as pspool,
        ):
            blk0 = 0
            for slot, nb in runs:
                # Weight quarters [128, 512], one per matmul rhs slice, emitted
                # n-major: the first 4 MiB (n=0 column set) unlocks every
                # token block's n=0 PSUM group ~12us in, instead of the whole
                # 16 MiB expert load (~45us) gating the first block.
                wt = {}
                for n in range(NTILES):
                    for k in range(KT):
                        t = wpool.tile(
                            [PB, NT], f32r, name=f"w_s{slot}_k{k}_n{n}", tag="w"
                        )
                        nc.sync.dma_start(
                            out=t,
                            in_=w[
                                slot,
                                k * PB : (k + 1) * PB,
                                n * NT : (n + 1) * NT,
                            ],
                        )
                        wt[(k, n)] = t
                b = 0
                while b < nb:
                    g = min(MG_BLOCKS, nb - b)
                    t0 = (blk0 + b) * PB
                    xt = xpool.tile(
                        [PB, KT, MG_BLOCKS * PB], f32r, name=f"xt_{blk0 + b}", tag="xt"
                    )[:, :, : g * PB]
                    # ACT HWDGE ring: keeps x/out traffic from queueing behind
                    # the 16 MiB expert weight streams on the SP ring.
                    nc.scalar.dma_start(
                        out=xt,
                        in_=xT[:, t0 : t0 + g * PB].rearrange(
                            "(k p) t -> p k t", p=PB
                        ),
                    )
                    for mb in range(g):
                        ot = opool.tile(
                            [PB, D_OUT], f32, name=f"o_{blk0 + b + mb}", tag="o"
                        )
                        for n in range(NTILES):
                            ps = pspool.tile([PB, NT], f32, name="ps", tag="ps")
                            for k in range(KT):
                                nc.tensor.matmul(
                                    ps,
                                    xt[:, k, mb * PB : (mb + 1) * PB],
                                    wt[(k, n)],
                                    start=(k == 0),
                                    stop=(k == KT - 1),
                                )
                            nc.vector.tensor_copy(
                                out=ot[:, n * NT : (n + 1) * NT], in_=ps
                            )
                        row = t0 + mb * PB
                        nc.scalar.dma_start(out=out[row : row + PB, :], in_=ot)
                    b += g
                blk0 += nb
    nc.compile()
    return nc


def kernel(x, weight, num_inputs_per_group):
    global LAST_RESULTS
    x = np.ascontiguousarray(np.asarray(x, dtype=np.float32))
    weight = np.ascontiguousarray(np.asarray(weight, dtype=np.float32))
    seg = _seg_from_groups(num_inputs_per_group)

    # --- plan: map 128-token blocks to experts ---------------------------------
    aligned = N_TOK % PB == 0 and all(
        np.all(seg[i * PB : (i + 1) * PB] == seg[i * PB]) for i in range(N_TOK // PB)
    )
    if aligned:
        block_expert = seg[:: PB].astype(np.int64)  # (256,)
        block_tokens = None  # identity: block b covers rows [b*128, (b+1)*128)
        n_blocks = len(block_expert)
    else:
        # Generic fallback: pad each contiguous expert segment to a 128 multiple
        # via a host-side gather; output rows are scattered back afterwards.
        bounds = np.flatnonzero(np.diff(seg)) + 1
        starts = np.concatenate([[0], bounds])
        ends = np.concatenate([bounds, [N_TOK]])
        blocks, experts = [], []
        for s, e in zip(starts, ends):
            idx = np.arange(s, e, dtype=np.int64)
            padded = -np.ones(int(np.ceil(len(idx) / PB)) * PB, dtype=np.int64)
            padded[: len(idx)] = idx
            for b0 in range(0, len(padded), PB):
                blocks.append(padded[b0 : b0 + PB])
                experts.append(int(seg[s]))
        while len(blocks) % NCORES:
            blocks.append(-np.ones(PB, dtype=np.int64))
            experts.append(0)
        block_tokens = np.stack(blocks)  # (n_blocks, 128) token ids, -1 = pad
        block_expert = np.asarray(experts, dtype=np.int64)
        n_blocks = len(block_expert)

    n_blocks_core = n_blocks // NCORES
    per_core_experts = block_expert.reshape(NCORES, n_blocks_core)

    # Run-length encode each core's block->expert map; if all cores share the
    # same run-length pattern we can use compact per-run weight slots.
    def rle(v):
        runs = []
        for e in v:
            if runs and runs[-1][0] == e:
                runs[-1][1] += 1
            else:
                runs.append([int(e), 1])
        return runs

    core_runs = [rle(per_core_experts[c]) for c in range(NCORES)]
    lengths0 = [n for _, n in core_runs[0]]
    if all([n for _, n in core_runs[c]] == lengths0 for c in range(NCORES)):
        runs = [(s, n) for s, (_, n) in enumerate(core_runs[0])]
        slot_experts = [[e for e, _ in core_runs[c]] for c in range(NCORES)]
    else:
        runs = [(b, 1) for b in range(n_blocks_core)]
        slot_experts = [list(per_core_experts[c]) for c in range(NCORES)]
    n_slots = len(runs)

    # --- per-core inputs -------------------------------------------------------
    in_maps = []
    for c in range(NCORES):
        if block_tokens is None:
            rows = slice(c * n_blocks_core * PB, (c + 1) * n_blocks_core * PB)
            xc = x[rows]
        else:
            tok = block_tokens[c * n_blocks_core : (c + 1) * n_blocks_core].ravel()
            xc = np.where(tok[:, None] >= 0, x[np.maximum(tok, 0)], 0.0).astype(
                np.float32
            )
        in_maps.append(
            {
                "xT": np.ascontiguousarray(xc.T),
                "w": np.ascontiguousarray(weight[slot_experts[c]]),
            }
        )

    nc = _build_nc(n_blocks_core, runs, n_slots)
    res = run_bass_kernel_spmd(
        nc, in_maps, core_ids=list(range(NCORES)), trace=TRACE
    )
    LAST_RESULTS = res

    # --- unshard ---------------------------------------------------------------
    outs = [res.results[c]["out"] for c in range(NCORES)]
    if block_tokens is None:
        return np.concatenate(outs, axis=0)
    full = np.zeros((N_TOK, D_OUT), dtype=np.float32)
    flat_tok = block_tokens.ravel()
    flat_out = np.concatenate(outs, axis=0)
    valid = flat_tok >= 0
    full[flat_tok[valid]] = flat_out[valid]
    return full
